# revision 1
# baseline (speedup 1.0000x reference)
"""Trainium2 Bass kernel for additive attention (nn_Attention_68968584839415).

Reference math:
    score[b,i,j] = <qry[b,i], w_q> + <key[b,j], w_k>
    att = softmax(score, axis=-1);  out = att @ val

Since softmax is shift-invariant along the reduced axis and the
<qry[b,i], w_q> term is constant in j, it cancels:
    att[b,i,:] = softmax(key[b] @ w_k) =: p[b]   (independent of i!)
    out[b,i,:] = p[b] @ val[b]        =: o[b]    (independent of i!)

So the device work is: tiny per-batch softmax + a 264 MiB broadcast-write
of the outputs — a pure HBM-write-bandwidth problem.

Sharding over 8 NeuronCores: core c handles (batch b = c//2, query-row
half h = c%2). Each core computes p[b]/o[b] from its copy of key[b],
val[b], w_k and writes att[b, h*2048:(h+1)*2048, :] (32 MiB, rows all
equal p[b]) and out[b, h*2048:(h+1)*2048, :] (1 MiB, rows all o[b]).
No collectives needed.
"""

from contextlib import ExitStack

import numpy as np

B, LQ, LK, D = 4, 4096, 4096, 128
LQ_HALF = LQ // 2
T = LK // 128  # 32 column-tiles of key/val
ROWBLK = LQ_HALF // 128  # 16 output row blocks per core
N_CORES = 8

_CACHE: dict = {}


def _build_graph():
    import concourse.bass as bass
    import concourse.mybir as mybir
    import concourse.tile as tile
    from concourse import bacc, bass_isa
    from concourse.masks import make_identity

    F32 = mybir.dt.float32

    nc = bacc.Bacc("TRN2", target_bir_lowering=False, debug=False)
    keyb = nc.dram_tensor("keyb", [LK, D], F32, kind="ExternalInput")
    valb = nc.dram_tensor("valb", [LK, D], F32, kind="ExternalInput")
    wk = nc.dram_tensor("wk", [D, 1], F32, kind="ExternalInput")
    att_part = nc.dram_tensor("att_part", [LQ_HALF, LK], F32, kind="ExternalOutput")
    out_part = nc.dram_tensor("out_part", [LQ_HALF, D], F32, kind="ExternalOutput")

    with ExitStack() as ctx:
        tc = ctx.enter_context(tile.TileContext(nc))
        singles = ctx.enter_context(tc.tile_pool(name="singles", bufs=1))
        psum_small = ctx.enter_context(
            tc.tile_pool(name="psum_small", bufs=1, space="PSUM")
        )
        psum_bc = ctx.enter_context(tc.tile_pool(name="psum_bc", bufs=4, space="PSUM"))

        # ---- constants
        identity = singles.tile([128, 128], F32)
        make_identity(nc, identity)
        ones_row = singles.tile([1, 128], F32)
        nc.vector.memset(ones_row, 1.0)

        # ---- load inputs
        # key_sb[p, t, d] = keyb[t*128 + p, d]
        key_sb = singles.tile([128, T, D], F32)
        nc.sync.dma_start(
            out=key_sb, in_=keyb.ap().rearrange("(t p) d -> p t d", p=128)
        )
        val_sb = singles.tile([128, T, D], F32)
        nc.sync.dma_start(
            out=val_sb, in_=valb.ap().rearrange("(t p) d -> p t d", p=128)
        )
        # wkb[p, d] = wk[d]  (broadcast along partitions via stride-0 AP)
        wkb = singles.tile([128, D], F32)
        wk_flat = wk.ap()
        wk_bcast = bass.AP(tensor=wk_flat.tensor, offset=0, ap=[[0, 128], [1, D]])
        nc.sync.dma_start(out=wkb, in_=wk_bcast)

        # ---- sk[p, t] = sum_d key_sb[p, t, d] * wk[d]
        prod = singles.tile([128, T, D], F32)
        wkb_ap = wkb[:]
        wkb3 = bass.AP(
            tensor=wkb_ap.tensor,
            offset=wkb_ap.offset,
            ap=[wkb_ap.ap[0], [0, T], wkb_ap.ap[1]],
        )
        nc.vector.tensor_mul(prod, key_sb, wkb3)
        sk_all = singles.tile([128, T], F32)
        nc.vector.reduce_sum(out=sk_all, in_=prod, axis=mybir.AxisListType.X)

        # ---- softmax over the whole [128, T] tile
        m1 = singles.tile([128, 1], F32)
        nc.vector.reduce_max(out=m1, in_=sk_all, axis=mybir.AxisListType.X)
        m_all = singles.tile([128, 1], F32)
        nc.gpsimd.partition_all_reduce(
            m_all, m1, channels=128, reduce_op=bass_isa.ReduceOp.max
        )
        nm = singles.tile([128, 1], F32)
        nc.vector.tensor_scalar_mul(nm, m_all, -1.0)

        e_all = singles.tile([128, T], F32)
        s1 = singles.tile([128, 1], F32)
        nc.scalar.activation(
            out=e_all,
            in_=sk_all,
            func=mybir.ActivationFunctionType.Exp,
            bias=nm,
            scale=1.0,
            accum_out=s1,
        )
        s_all = singles.tile([128, 1], F32)
        nc.gpsimd.partition_all_reduce(
            s_all, s1, channels=128, reduce_op=bass_isa.ReduceOp.add
        )
        rinv = singles.tile([128, 1], F32)
        nc.vector.reciprocal(rinv, s_all)

        # ---- o = (1/S) * (e @ val):  psum_o[1, D] accumulated over T tiles
        psum_o = psum_small.tile([1, D], F32)
        for t in range(T):
            nc.tensor.matmul(
                psum_o,
                lhsT=e_all[:, t : t + 1],
                rhs=val_sb[:, t, :],
                start=(t == 0),
                stop=(t == T - 1),
            )
        o_row = singles.tile([1, D], F32)
        nc.scalar.activation(
            out=o_row,
            in_=psum_o,
            func=mybir.ActivationFunctionType.Copy,
            bias=0.0,
            scale=rinv[0:1, :],
        )

        # ---- e_row [1, LK]: transpose e_all -> [T, 128] -> flatten via DMA
        psum_t = psum_small.tile([T, 128], F32)
        nc.tensor.transpose(psum_t, e_all, identity)
        e32 = singles.tile([T, 128], F32)
        nc.scalar.copy(e32, psum_t)
        e_row = singles.tile([1, LK], F32)
        nc.sync.dma_start(out=e_row, in_=e32)

        # sinv_row [1, 128]: every element = 1/S (normalization fused into the
        # broadcast matmul below)
        sinv_row = singles.tile([1, 128], F32)
        r_ap = rinv[0:1, :]
        r_bcast = bass.AP(
            tensor=r_ap.tensor, offset=r_ap.offset, ap=[r_ap.ap[0], [0, 128]]
        )
        nc.vector.tensor_copy(sinv_row, r_bcast)

        # ---- p_rep [128, LK]: every partition = p = e_row / S
        p_rep = singles.tile([128, LK], F32)
        NCHUNK = 512
        for n in range(LK // NCHUNK):
            pb = psum_bc.tile([128, NCHUNK], F32, name=f"pb{n}", tag="pb")
            nc.tensor.matmul(
                pb,
                lhsT=sinv_row,
                rhs=e_row[:, n * NCHUNK : (n + 1) * NCHUNK],
                start=True,
                stop=True,
            )
            nc.vector.tensor_copy(p_rep[:, n * NCHUNK : (n + 1) * NCHUNK], pb)

        # ---- o_rep [128, D] = ones^T @ o_row
        psum_ob = psum_small.tile([128, D], F32)
        nc.tensor.matmul(psum_ob, lhsT=ones_row, rhs=o_row, start=True, stop=True)
        o_rep = singles.tile([128, D], F32)
        nc.vector.tensor_copy(o_rep, psum_ob)

        # ---- output writes (the memory-bound part: 33 MiB per core)
        att_ap = att_part.ap()
        out_ap = out_part.ap()
        for k in range(ROWBLK):
            nc.sync.dma_start(out=att_ap[k * 128 : (k + 1) * 128, :], in_=p_rep)
        for k in range(ROWBLK):
            nc.scalar.dma_start(out=out_ap[k * 128 : (k + 1) * 128, :], in_=o_rep)

    nc.compile()
    return nc


def _get_graph():
    if "nc" not in _CACHE:
        _CACHE["nc"] = _build_graph()
    return _CACHE["nc"]


def kernel(qry=None, key=None, val=None, w_q=None, w_k=None, **_ignored):
    """Full (unsharded) inputs in, full outputs out.

    Returns (out, att) matching reference.reference(). qry/w_q are
    mathematically irrelevant (softmax shift invariance) and unused.
    """
    from concourse.bass_utils import run_bass_kernel_spmd

    key = np.ascontiguousarray(np.asarray(key, dtype=np.float32))
    val = np.ascontiguousarray(np.asarray(val, dtype=np.float32))
    w_k = np.ascontiguousarray(np.asarray(w_k, dtype=np.float32))

    nc = _get_graph()
    in_maps = []
    for c in range(N_CORES):
        b = c // 2
        in_maps.append({"keyb": key[b], "valb": val[b], "wk": w_k})

    res = run_bass_kernel_spmd(nc, in_maps, core_ids=list(range(N_CORES)))

    out = np.empty((B, LQ, D), np.float32)
    att = np.empty((B, LQ, LK), np.float32)
    for c in range(N_CORES):
        b, h = c // 2, c % 2
        att[b, h * LQ_HALF : (h + 1) * LQ_HALF, :] = res.results[c]["att_part"]
        out[b, h * LQ_HALF : (h + 1) * LQ_HALF, :] = res.results[c]["out_part"]
    return out, att


# revision 2
# speedup vs baseline: 1.0266x; 1.0266x over previous
"""Trainium2 Bass kernel for additive attention (nn_Attention_68968584839415).

Reference math:
    score[b,i,j] = <qry[b,i], w_q> + <key[b,j], w_k>
    att = softmax(score, axis=-1);  out = att @ val

Since softmax is shift-invariant along the reduced axis and the
<qry[b,i], w_q> term is constant in j, it cancels:
    att[b,i,:] = softmax(key[b] @ w_k) =: p[b]   (independent of i!)
    out[b,i,:] = p[b] @ val[b]        =: o[b]    (independent of i!)

So the device work is a tiny per-batch softmax + a 264 MiB broadcast-write
of the outputs — a pure HBM-write-bandwidth problem (~92 us per core for
its 33 MiB share at ~360 GB/s).

Sharding over 8 NeuronCores: core c handles (batch b = c//2, query-row
half h = c%2). Each core computes p[b]/o[b] from its copy of key[b],
val[b], w_k and writes att[b, h*2048:(h+1)*2048, :] (32 MiB, rows all
equal p[b]) and out[b, h*2048:(h+1)*2048, :] (1 MiB, rows all o[b]).
No collectives needed.

Schedule: the only work gating the big att-write stream is
key load -> sk = key @ w_k -> softmax -> p broadcast to [128, LK].
Everything else (val load, o = p @ val, out_part writes) is issued after
the att DMAs and overlaps the stream.
"""

from contextlib import ExitStack

import numpy as np

B, LQ, LK, D = 4, 4096, 4096, 128
LQ_HALF = LQ // 2
T = LK // 128  # 32 column-tiles of key/val
ROWBLK = LQ_HALF // 128  # 16 output row blocks per core
N_CORES = 8

# p-broadcast implementation: "gpsimd" (partition_broadcast) or "matmul"
BCAST_IMPL = "gpsimd"

_CACHE: dict = {}


def _build_graph():
    import concourse.bass as bass
    import concourse.mybir as mybir
    import concourse.tile as tile
    from concourse import bacc, bass_isa
    from concourse.masks import make_identity

    F32 = mybir.dt.float32

    nc = bacc.Bacc("TRN2", target_bir_lowering=False, debug=False)
    keyb = nc.dram_tensor("keyb", [LK, D], F32, kind="ExternalInput")
    valb = nc.dram_tensor("valb", [LK, D], F32, kind="ExternalInput")
    wk = nc.dram_tensor("wk", [D, 1], F32, kind="ExternalInput")
    att_part = nc.dram_tensor("att_part", [LQ_HALF, LK], F32, kind="ExternalOutput")
    out_part = nc.dram_tensor("out_part", [LQ_HALF, D], F32, kind="ExternalOutput")

    with ExitStack() as ctx:
        tc = ctx.enter_context(tile.TileContext(nc))
        singles = ctx.enter_context(tc.tile_pool(name="singles", bufs=1))
        psum_small = ctx.enter_context(
            tc.tile_pool(name="psum_small", bufs=1, space="PSUM")
        )
        psum_bc = ctx.enter_context(tc.tile_pool(name="psum_bc", bufs=4, space="PSUM"))

        # ---- constants / input loads on the critical path
        identity = singles.tile([128, 128], F32)
        make_identity(nc, identity)
        ones_row = singles.tile([1, 128], F32)
        nc.vector.memset(ones_row, 1.0)

        # wkb[p, d] = wk[d]  (tiny partition-broadcast DMA from DRAM)
        wkb = singles.tile([128, D], F32)
        wk_flat = wk.ap()
        wk_bcast = bass.AP(tensor=wk_flat.tensor, offset=0, ap=[[0, 128], [1, D]])
        nc.sync.dma_start(out=wkb, in_=wk_bcast)
        # key_sb[p, t, d] = keyb[t*128 + p, d]
        key_sb = singles.tile([128, T, D], F32)
        nc.sync.dma_start(
            out=key_sb, in_=keyb.ap().rearrange("(t p) d -> p t d", p=128)
        )

        # wkb_full[p, t*D + d] = wk[d]: materialized (runs during the key DMA)
        # so the multiply reads unit-stride operands.
        wkb_full = singles.tile([128, T, D], F32)
        wkb_ap = wkb[:]
        wkb3 = bass.AP(
            tensor=wkb_ap.tensor,
            offset=wkb_ap.offset,
            ap=[wkb_ap.ap[0], [0, T], wkb_ap.ap[1]],
        )
        nc.vector.tensor_copy(wkb_full, wkb3)

        # ---- sk[p, t] = sum_d key_sb[p, t, d] * wk[d]
        prod = singles.tile([128, T, D], F32)
        nc.vector.tensor_mul(prod, key_sb, wkb_full)
        sk_all = singles.tile([128, T], F32)
        nc.vector.reduce_sum(out=sk_all, in_=prod, axis=mybir.AxisListType.X)

        # ---- softmax over the whole [128, T] tile, fully normalized via a
        # second exp: p = exp(sk - M - ln S) where S = sum exp(sk - M).
        m1 = singles.tile([128, 1], F32)
        nc.vector.reduce_max(out=m1, in_=sk_all, axis=mybir.AxisListType.X)
        m_all = singles.tile([128, 1], F32)
        nc.gpsimd.partition_all_reduce(
            m_all, m1, channels=128, reduce_op=bass_isa.ReduceOp.max
        )
        nm = singles.tile([128, 1], F32)
        nc.vector.tensor_scalar_mul(nm, m_all, -1.0)

        e_all = singles.tile([128, T], F32)
        s1 = singles.tile([128, 1], F32)
        nc.scalar.activation(
            out=e_all,
            in_=sk_all,
            func=mybir.ActivationFunctionType.Exp,
            bias=nm,
            scale=1.0,
            accum_out=s1,
        )
        s_all = singles.tile([128, 1], F32)
        nc.gpsimd.partition_all_reduce(
            s_all, s1, channels=128, reduce_op=bass_isa.ReduceOp.add
        )
        ln_s = singles.tile([128, 1], F32)
        nc.scalar.activation(
            out=ln_s, in_=s_all, func=mybir.ActivationFunctionType.Ln
        )
        mls = singles.tile([128, 1], F32)
        nc.vector.tensor_add(mls, m_all, ln_s)
        nmls = singles.tile([128, 1], F32)
        nc.vector.tensor_scalar_mul(nmls, mls, -1.0)
        p_all = singles.tile([128, T], F32)
        nc.scalar.activation(
            out=p_all,
            in_=sk_all,
            func=mybir.ActivationFunctionType.Exp,
            bias=nmls,
            scale=1.0,
        )

        # ---- p_row [1, LK]: transpose p_all -> [T, 128] -> flatten via DMA
        psum_t = psum_small.tile([T, 128], F32)
        nc.tensor.transpose(psum_t, p_all, identity)
        p32 = singles.tile([T, 128], F32)
        nc.vector.tensor_copy(p32, psum_t)
        p_row = singles.tile([1, LK], F32)
        nc.sync.dma_start(out=p_row, in_=p32)

        # ---- p_rep [128, LK]: every partition = p
        p_rep = singles.tile([128, LK], F32)
        if BCAST_IMPL == "gpsimd":
            nc.gpsimd.partition_broadcast(p_rep, p_row)
        else:
            NCHUNK = 512
            for n in range(LK // NCHUNK):
                pb = psum_bc.tile([128, NCHUNK], F32, name=f"pb{n}", tag="pb")
                nc.tensor.matmul(
                    pb,
                    lhsT=ones_row,
                    rhs=p_row[:, n * NCHUNK : (n + 1) * NCHUNK],
                    start=True,
                    stop=True,
                )
                nc.vector.tensor_copy(p_rep[:, n * NCHUNK : (n + 1) * NCHUNK], pb)

        # ---- THE stream: 32 MiB of att writes (16 x 2 MiB, sync HWDGE ring)
        att_ap = att_part.ap()
        for k in range(ROWBLK):
            nc.sync.dma_start(out=att_ap[k * 128 : (k + 1) * 128, :], in_=p_rep)

        # ---- everything below overlaps the att stream
        # val load via SWDGE (gpsimd) so it can't delay anything above
        val_sb = singles.tile([128, T, D], F32)
        nc.gpsimd.dma_start(
            out=val_sb, in_=valb.ap().rearrange("(t p) d -> p t d", p=128)
        )

        # o = p @ val: psum_o[1, D] accumulated over T tiles
        psum_o = psum_small.tile([1, D], F32)
        for t in range(T):
            nc.tensor.matmul(
                psum_o,
                lhsT=p_all[:, t : t + 1],
                rhs=val_sb[:, t, :],
                start=(t == 0),
                stop=(t == T - 1),
            )
        o_row = singles.tile([1, D], F32)
        nc.scalar.copy(o_row, psum_o)

        # o_rep [128, D] = ones^T @ o_row
        psum_ob = psum_small.tile([128, D], F32)
        nc.tensor.matmul(psum_ob, lhsT=ones_row, rhs=o_row, start=True, stop=True)
        o_rep = singles.tile([128, D], F32)
        nc.vector.tensor_copy(o_rep, psum_ob)

        # out_part writes (16 x 64 KiB) on the scalar HWDGE ring
        out_ap = out_part.ap()
        for k in range(ROWBLK):
            nc.scalar.dma_start(out=out_ap[k * 128 : (k + 1) * 128, :], in_=o_rep)

    nc.compile()
    return nc


def _get_graph():
    if "nc" not in _CACHE:
        _CACHE["nc"] = _build_graph()
    return _CACHE["nc"]


def kernel(qry=None, key=None, val=None, w_q=None, w_k=None, **_ignored):
    """Full (unsharded) inputs in, full outputs out.

    Returns (out, att) matching reference.reference(). qry/w_q are
    mathematically irrelevant (softmax shift invariance) and unused.
    """
    from concourse.bass_utils import run_bass_kernel_spmd

    key = np.ascontiguousarray(np.asarray(key, dtype=np.float32))
    val = np.ascontiguousarray(np.asarray(val, dtype=np.float32))
    w_k = np.ascontiguousarray(np.asarray(w_k, dtype=np.float32))

    nc = _get_graph()
    in_maps = []
    for c in range(N_CORES):
        b = c // 2
        in_maps.append({"keyb": key[b], "valb": val[b], "wk": w_k})

    res = run_bass_kernel_spmd(nc, in_maps, core_ids=list(range(N_CORES)))

    out = np.empty((B, LQ, D), np.float32)
    att = np.empty((B, LQ, LK), np.float32)
    for c in range(N_CORES):
        b, h = c // 2, c % 2
        att[b, h * LQ_HALF : (h + 1) * LQ_HALF, :] = res.results[c]["att_part"]
        out[b, h * LQ_HALF : (h + 1) * LQ_HALF, :] = res.results[c]["out_part"]
    return out, att


# revision 3
# speedup vs baseline: 1.1171x; 1.0882x over previous
"""Trainium2 Bass kernel for additive attention (nn_Attention_68968584839415).

Reference math:
    score[b,i,j] = <qry[b,i], w_q> + <key[b,j], w_k>
    att = softmax(score, axis=-1);  out = att @ val

Since softmax is shift-invariant along the reduced axis and the
<qry[b,i], w_q> term is constant in j, it cancels:
    att[b,i,:] = softmax(key[b] @ w_k) =: p[b]   (independent of i!)
    out[b,i,:] = p[b] @ val[b]        =: o[b]    (independent of i!)

So the device work is a tiny per-batch softmax + a 264 MiB broadcast-write
of the outputs — a pure HBM-write-bandwidth problem (~92 us per core for
its 33 MiB share at ~360 GB/s).

Sharding over 8 NeuronCores: core c handles (batch b = c//2, query-row
half h = c%2). Each core computes p[b]/o[b] from its copy of key[b],
val[b], w_k and writes att[b, h*2048:(h+1)*2048, :] (32 MiB, rows all
equal p[b]) and out[b, h*2048:(h+1)*2048, :] (1 MiB, rows all o[b]).
No collectives needed.

Layout trick: key/val are loaded contiguously as [128, R=32, D] with
j = p*R + r (partition-major), so the flatten of the per-j scores IS j
order — p_row [1, LK] comes straight out of a tiny SBUF->SBUF DMA, no
transpose. Loads are 128 x 16 KiB descriptors (line rate).

Critical path: key load -> sk = key.wk (DVE mult+reduce) -> softmax ->
p_row -> partition_broadcast (2 halves) -> 32 MiB att stream.
val load, o = p @ val, and out_part writes overlap the stream.
"""

from contextlib import ExitStack

import numpy as np

B, LQ, LK, D = 4, 4096, 4096, 128
LQ_HALF = LQ // 2
R = LK // 128  # 32 rows per partition in the contiguous load
ROWBLK = LQ_HALF // 128  # 16 output row blocks per core
N_CORES = 8

# Use fp16 for the sk = key @ w_k arithmetic (2x DVE throughput; ~5e-4
# relative error on p, far inside the 2e-2 gate). f32 otherwise.
FP16_SK = False

_CACHE: dict = {}


def _build_graph():
    import concourse.bass as bass
    import concourse.mybir as mybir
    import concourse.tile as tile
    from concourse import bacc, bass_isa
    from concourse.tile import add_dep_helper

    F32 = mybir.dt.float32
    CDT = mybir.dt.float16 if FP16_SK else F32

    nc = bacc.Bacc("TRN2", target_bir_lowering=False, debug=False)
    keyb = nc.dram_tensor("keyb", [LK, D], F32, kind="ExternalInput")
    valb = nc.dram_tensor("valb", [LK, D], F32, kind="ExternalInput")
    wk = nc.dram_tensor("wk", [D, 1], F32, kind="ExternalInput")
    att_part = nc.dram_tensor("att_part", [LQ_HALF, LK], F32, kind="ExternalOutput")
    out_part = nc.dram_tensor("out_part", [LQ_HALF, D], F32, kind="ExternalOutput")

    with ExitStack() as ctx:
        tc = ctx.enter_context(tile.TileContext(nc))
        singles = ctx.enter_context(tc.tile_pool(name="singles", bufs=1))
        psum_small = ctx.enter_context(
            tc.tile_pool(name="psum_small", bufs=1, space="PSUM")
        )

        # ---- critical path: key load + wk broadcast
        # wkb[p, d] = wk[d]  (tiny partition-broadcast DMA from DRAM)
        wkb = singles.tile([128, D], F32)
        wk_flat = wk.ap()
        wk_bcast = bass.AP(tensor=wk_flat.tensor, offset=0, ap=[[0, 128], [1, D]])
        nc.sync.dma_start(out=wkb, in_=wk_bcast)

        # key_sb[p, r, d] = keyb[p*R + r, d]: contiguous 16 KiB per partition
        if FP16_SK:
            key_sb = singles.tile([128, R, D], CDT)
            nc.gpsimd.dma_start(  # SWDGE: casts f32 -> fp16 in flight
                out=key_sb, in_=keyb.ap().rearrange("(p r) d -> p r d", p=128)
            )
        else:
            key_sb = singles.tile([128, R, D], F32)
            nc.sync.dma_start(
                out=key_sb, in_=keyb.ap().rearrange("(p r) d -> p r d", p=128)
            )

        # wkb_full[p, r, d] = wk[d], materialized during the key DMA
        wkb_full = singles.tile([128, R, D], CDT)
        wkb_ap = wkb[:]
        wkb3 = bass.AP(
            tensor=wkb_ap.tensor,
            offset=wkb_ap.offset,
            ap=[wkb_ap.ap[0], [0, R], wkb_ap.ap[1]],
        )
        nc.vector.tensor_copy(wkb_full, wkb3)

        # ---- sk[p, r] = sum_d key_sb[p, r, d] * wk[d]
        prod = singles.tile([128, R, D], CDT)
        nc.vector.tensor_mul(prod, key_sb, wkb_full)
        sk_all = singles.tile([128, R], CDT)
        red_inst = nc.vector.reduce_sum(out=sk_all, in_=prod, axis=mybir.AxisListType.X)

        # ---- softmax over the whole [128, R] tile
        m1 = singles.tile([128, 1], F32)
        nc.vector.reduce_max(out=m1, in_=sk_all, axis=mybir.AxisListType.X)
        m_all = singles.tile([128, 1], F32)
        nc.gpsimd.partition_all_reduce(
            m_all, m1, channels=128, reduce_op=bass_isa.ReduceOp.max
        )
        nm = singles.tile([128, 1], F32)
        nc.vector.tensor_scalar_mul(nm, m_all, -1.0)

        e_all = singles.tile([128, R], F32)
        s1 = singles.tile([128, 1], F32)
        nc.scalar.activation(
            out=e_all,
            in_=sk_all,
            func=mybir.ActivationFunctionType.Exp,
            bias=nm,
            scale=1.0,
            accum_out=s1,
        )
        s_all = singles.tile([128, 1], F32)
        nc.gpsimd.partition_all_reduce(
            s_all, s1, channels=128, reduce_op=bass_isa.ReduceOp.add
        )
        rinv = singles.tile([128, 1], F32)
        nc.vector.reciprocal(rinv, s_all)
        p_all = singles.tile([128, R], F32)
        nc.vector.tensor_scalar_mul(p_all, e_all, rinv)

        # ---- p_row [1, LK]: partition-major flatten of p_all IS j order
        p_row = singles.tile([1, LK], F32)
        nc.sync.dma_start(out=p_row, in_=p_all)

        # ---- p_rep [128, LK] in two halves; att stream starts after half 0
        p_rep = singles.tile([128, LK], F32)
        HALF = LK // 2
        att_ap = att_part.ap()
        for hh in range(2):
            cs = slice(hh * HALF, (hh + 1) * HALF)
            nc.gpsimd.partition_broadcast(p_rep[:, cs], p_row[:, cs])
            for k in range(ROWBLK):
                nc.sync.dma_start(
                    out=att_ap[k * 128 : (k + 1) * 128, cs], in_=p_rep[:, cs]
                )

        # ---- everything below overlaps the att stream
        # val load (contiguous, SWDGE), held off key's HBM window
        val_sb = singles.tile([128, R, D], F32)
        vdma = nc.gpsimd.dma_start(
            out=val_sb, in_=valb.ap().rearrange("(p r) d -> p r d", p=128)
        )
        add_dep_helper(
            getattr(vdma, "ins", vdma),
            getattr(red_inst, "ins", red_inst),
            reason="delay val load off key's HBM read window",
        )

        # o = p @ val: psum_o[1, D] accumulated over R column-tiles
        psum_o = psum_small.tile([1, D], F32)
        for r in range(R):
            nc.tensor.matmul(
                psum_o,
                lhsT=p_all[:, r : r + 1],
                rhs=val_sb[:, r, :],
                start=(r == 0),
                stop=(r == R - 1),
            )
        o_row = singles.tile([1, D], F32)
        nc.scalar.copy(o_row, psum_o)

        # o_rep [128, D]: broadcast o to all partitions
        o_rep = singles.tile([128, D], F32)
        nc.gpsimd.partition_broadcast(o_rep, o_row)

        # out_part writes (16 x 64 KiB) on the scalar HWDGE ring
        out_ap = out_part.ap()
        for k in range(ROWBLK):
            nc.scalar.dma_start(out=out_ap[k * 128 : (k + 1) * 128, :], in_=o_rep)

    nc.compile()
    return nc


def _get_graph():
    if "nc" not in _CACHE:
        _CACHE["nc"] = _build_graph()
    return _CACHE["nc"]


def kernel(qry=None, key=None, val=None, w_q=None, w_k=None, **_ignored):
    """Full (unsharded) inputs in, full outputs out.

    Returns (out, att) matching reference.reference(). qry/w_q are
    mathematically irrelevant (softmax shift invariance) and unused.
    """
    from concourse.bass_utils import run_bass_kernel_spmd

    key = np.ascontiguousarray(np.asarray(key, dtype=np.float32))
    val = np.ascontiguousarray(np.asarray(val, dtype=np.float32))
    w_k = np.ascontiguousarray(np.asarray(w_k, dtype=np.float32))

    nc = _get_graph()
    in_maps = []
    for c in range(N_CORES):
        b = c // 2
        in_maps.append({"keyb": key[b], "valb": val[b], "wk": w_k})

    res = run_bass_kernel_spmd(nc, in_maps, core_ids=list(range(N_CORES)))

    out = np.empty((B, LQ, D), np.float32)
    att = np.empty((B, LQ, LK), np.float32)
    for c in range(N_CORES):
        b, h = c // 2, c % 2
        att[b, h * LQ_HALF : (h + 1) * LQ_HALF, :] = res.results[c]["att_part"]
        out[b, h * LQ_HALF : (h + 1) * LQ_HALF, :] = res.results[c]["out_part"]
    return out, att


# revision 6
# speedup vs baseline: 1.1967x; 1.0712x over previous
"""Trainium2 Bass kernel for additive attention (nn_Attention_68968584839415).

Reference math:
    score[b,i,j] = <qry[b,i], w_q> + <key[b,j], w_k>
    att = softmax(score, axis=-1);  out = att @ val

Since softmax is shift-invariant along the reduced axis and the
<qry[b,i], w_q> term is constant in j, it cancels:
    att[b,i,:] = softmax(key[b] @ w_k) =: p[b]   (independent of i!)
    out[b,i,:] = p[b] @ val[b]        =: o[b]    (independent of i!)

So the device work is a tiny per-batch softmax + a 264 MiB broadcast-write
of the outputs — a pure HBM-write-bandwidth problem (~92 us per core for
its 33 MiB share at ~360 GB/s).

Sharding over 8 NeuronCores: core c handles (batch b = c//2, query-row
half h = c%2). Each core computes p[b]/o[b] from its copy of key[b],
val[b], w_k and writes att[b, h*2048:(h+1)*2048, :] (32 MiB, rows all
equal p[b]) and out[b, h*2048:(h+1)*2048, :] (1 MiB, rows all o[b]).
No collectives needed.

Layout trick: key/val are loaded contiguously as [128, R=32, D] with
j = p*R + r (partition-major), so the flatten of the per-j scores IS j
order — p_row [1, LK] comes straight out of a tiny SBUF->SBUF DMA, no
transpose. Loads are 128 x 16 KiB descriptors (line rate).

Critical path: key load -> sk = key.wk (DVE mult+reduce) -> softmax ->
p_row -> partition_broadcast (2 halves) -> 32 MiB att stream.
val load, o = p @ val, and out_part writes overlap the stream.
"""

from contextlib import ExitStack

import numpy as np

B, LQ, LK, D = 4, 4096, 4096, 128
LQ_HALF = LQ // 2
R = LK // 128  # 32 rows per partition in the contiguous load
ROWBLK = LQ_HALF // 128  # 16 output row blocks per core
N_CORES = 8

# Use fp16 for the sk = key @ w_k arithmetic (2x DVE throughput; ~5e-4
# relative error on p, far inside the 2e-2 gate). f32 otherwise.
FP16_SK = False

_CACHE: dict = {}


def _build_graph():
    import concourse.bass as bass
    import concourse.mybir as mybir
    import concourse.tile as tile
    from concourse import bacc, bass_isa
    from concourse.tile import add_dep_helper

    F32 = mybir.dt.float32
    CDT = mybir.dt.float16 if FP16_SK else F32

    nc = bacc.Bacc("TRN2", target_bir_lowering=False, debug=False)
    keyb = nc.dram_tensor("keyb", [LK, D], F32, kind="ExternalInput")
    valb = nc.dram_tensor("valb", [LK, D], F32, kind="ExternalInput")
    wk = nc.dram_tensor("wk", [D, 1], F32, kind="ExternalInput")
    att_part = nc.dram_tensor("att_part", [LQ_HALF, LK], F32, kind="ExternalOutput")
    out_part = nc.dram_tensor("out_part", [LQ_HALF, D], F32, kind="ExternalOutput")

    with ExitStack() as ctx:
        tc = ctx.enter_context(tile.TileContext(nc))
        singles = ctx.enter_context(tc.tile_pool(name="singles", bufs=1))
        psum_small = ctx.enter_context(
            tc.tile_pool(name="psum_small", bufs=1, space="PSUM")
        )

        # ---- critical path: key load + wk broadcast
        # key_sb[p, r, d] = keyb[p*R + r, d]: contiguous 16 KiB per partition
        if FP16_SK:
            key_sb = singles.tile([128, R, D], CDT)
            nc.gpsimd.dma_start(  # SWDGE: casts f32 -> fp16 in flight
                out=key_sb, in_=keyb.ap().rearrange("(p r) d -> p r d", p=128)
            )
        else:
            key_sb = singles.tile([128, R, D], F32)
            nc.sync.dma_start(
                out=key_sb, in_=keyb.ap().rearrange("(p r) d -> p r d", p=128)
            )

        # wkb[p, d] = wk[d]  (tiny partition-broadcast DMA from DRAM)
        wkb = singles.tile([128, D], F32)
        wk_flat = wk.ap()
        wk_bcast = bass.AP(tensor=wk_flat.tensor, offset=0, ap=[[0, 128], [1, D]])
        nc.sync.dma_start(out=wkb, in_=wk_bcast)

        # wkb_full[p, r, d] = wk[d], materialized during the key DMA
        wkb_full = singles.tile([128, R, D], CDT)
        wkb_ap = wkb[:]
        wkb3 = bass.AP(
            tensor=wkb_ap.tensor,
            offset=wkb_ap.offset,
            ap=[wkb_ap.ap[0], [0, R], wkb_ap.ap[1]],
        )
        nc.vector.tensor_copy(wkb_full, wkb3)

        # ---- sk[p, r] = sum_d key_sb[p, r, d] * wk[d]
        prod = singles.tile([128, R, D], CDT)
        nc.vector.tensor_mul(prod, key_sb, wkb_full)
        sk_all = singles.tile([128, R], CDT)
        red_inst = nc.vector.reduce_sum(out=sk_all, in_=prod, axis=mybir.AxisListType.X)

        # ---- softmax over the whole [128, R] tile
        m1 = singles.tile([128, 1], F32)
        nc.vector.reduce_max(out=m1, in_=sk_all, axis=mybir.AxisListType.X)
        m_all = singles.tile([128, 1], F32)
        nc.gpsimd.partition_all_reduce(
            m_all, m1, channels=128, reduce_op=bass_isa.ReduceOp.max
        )
        nm = singles.tile([128, 1], F32)
        nc.vector.tensor_scalar_mul(nm, m_all, -1.0)

        e_all = singles.tile([128, R], F32)
        s1 = singles.tile([128, 1], F32)
        nc.scalar.activation(
            out=e_all,
            in_=sk_all,
            func=mybir.ActivationFunctionType.Exp,
            bias=nm,
            scale=1.0,
            accum_out=s1,
        )
        s_all = singles.tile([128, 1], F32)
        nc.gpsimd.partition_all_reduce(
            s_all, s1, channels=128, reduce_op=bass_isa.ReduceOp.add
        )
        rinv = singles.tile([128, 1], F32)
        nc.vector.reciprocal(rinv, s_all)
        p_all = singles.tile([128, R], F32)
        nc.vector.tensor_scalar_mul(p_all, e_all, rinv)

        # ---- p_row [1, LK]: partition-major flatten of p_all IS j order
        p_row = singles.tile([1, LK], F32)
        nc.sync.dma_start(out=p_row, in_=p_all)

        # ---- p_rep [128, LK] in two halves; att stream starts after half 0
        p_rep = singles.tile([128, LK], F32)
        HALF = LK // 2
        att_ap = att_part.ap()
        last_pb = None
        for hh in range(2):
            cs = slice(hh * HALF, (hh + 1) * HALF)
            last_pb = nc.gpsimd.partition_broadcast(p_rep[:, cs], p_row[:, cs])
            for k in range(ROWBLK):
                nc.sync.dma_start(
                    out=att_ap[k * 128 : (k + 1) * 128, cs], in_=p_rep[:, cs]
                )

        # ---- everything below overlaps the att stream
        # val load on the scalar HWDGE ring (so the gpsimd queue never waits
        # on its completion), held off key's HBM read window
        val_sb = singles.tile([128, R, D], F32)
        vdma = nc.scalar.dma_start(
            out=val_sb, in_=valb.ap().rearrange("(p r) d -> p r d", p=128)
        )
        add_dep_helper(
            getattr(vdma, "ins", vdma),
            getattr(red_inst, "ins", red_inst),
            reason="delay val load off key's HBM read window",
        )

        # o = p @ val: psum_o[1, D] accumulated over R column-tiles
        psum_o = psum_small.tile([1, D], F32)
        for r in range(R):
            nc.tensor.matmul(
                psum_o,
                lhsT=p_all[:, r : r + 1],
                rhs=val_sb[:, r, :],
                start=(r == 0),
                stop=(r == R - 1),
            )
        o_row = singles.tile([1, D], F32)
        nc.scalar.copy(o_row, psum_o)

        # o_rep [128, D]: broadcast o to all partitions (explicitly ordered
        # after the p_rep broadcasts so it can't head-of-line block them)
        o_rep = singles.tile([128, D], F32)
        opb = nc.gpsimd.partition_broadcast(o_rep, o_row)
        add_dep_helper(
            getattr(opb, "ins", opb),
            getattr(last_pb, "ins", last_pb),
            reason="o_rep broadcast must not precede p_rep broadcasts",
        )

        # out_part writes (16 x 64 KiB) on the scalar HWDGE ring
        out_ap = out_part.ap()
        for k in range(ROWBLK):
            nc.scalar.dma_start(out=out_ap[k * 128 : (k + 1) * 128, :], in_=o_rep)

    nc.compile()
    return nc


def _get_graph():
    if "nc" not in _CACHE:
        _CACHE["nc"] = _build_graph()
    return _CACHE["nc"]


def kernel(qry=None, key=None, val=None, w_q=None, w_k=None, **_ignored):
    """Full (unsharded) inputs in, full outputs out.

    Returns (out, att) matching reference.reference(). qry/w_q are
    mathematically irrelevant (softmax shift invariance) and unused.
    """
    from concourse.bass_utils import run_bass_kernel_spmd

    key = np.ascontiguousarray(np.asarray(key, dtype=np.float32))
    val = np.ascontiguousarray(np.asarray(val, dtype=np.float32))
    w_k = np.ascontiguousarray(np.asarray(w_k, dtype=np.float32))

    nc = _get_graph()
    in_maps = []
    for c in range(N_CORES):
        b = c // 2
        in_maps.append({"keyb": key[b], "valb": val[b], "wk": w_k})

    res = run_bass_kernel_spmd(nc, in_maps, core_ids=list(range(N_CORES)))

    out = np.empty((B, LQ, D), np.float32)
    att = np.empty((B, LQ, LK), np.float32)
    for c in range(N_CORES):
        b, h = c // 2, c % 2
        att[b, h * LQ_HALF : (h + 1) * LQ_HALF, :] = res.results[c]["att_part"]
        out[b, h * LQ_HALF : (h + 1) * LQ_HALF, :] = res.results[c]["out_part"]
    return out, att


# revision 10
# speedup vs baseline: 1.2456x; 1.0409x over previous
"""Trainium2 Bass kernel for additive attention (nn_Attention_68968584839415).

Reference math:
    score[b,i,j] = <qry[b,i], w_q> + <key[b,j], w_k>
    att = softmax(score, axis=-1);  out = att @ val

Since softmax is shift-invariant along the reduced axis and the
<qry[b,i], w_q> term is constant in j, it cancels:
    att[b,i,:] = softmax(key[b] @ w_k) =: p[b]   (independent of i!)
    out[b,i,:] = p[b] @ val[b]        =: o[b]    (independent of i!)

So the device work is a tiny per-batch softmax + a 264 MiB broadcast-write
of the outputs — a pure HBM-write-bandwidth problem (~92 us per core for
its 33 MiB share at ~360 GB/s).

Sharding over 8 NeuronCores: core c handles (batch b = c//2, query-row
half h = c%2). Each core computes p[b]/o[b] from its copy of key[b],
val[b], w_k and writes att[b, h*2048:(h+1)*2048, :] (32 MiB, rows all
equal p[b]) and out[b, h*2048:(h+1)*2048, :] (1 MiB, rows all o[b]).
No collectives needed.

Layout trick: key/val are loaded contiguously as [128, R=32, D] with
j = p*R + r (partition-major), so the flatten of the per-j scores IS j
order — p_row [1, LK] comes straight out of a tiny SBUF->SBUF DMA, no
transpose. Loads are 128 x 16 KiB descriptors (line rate).

Critical path: key load -> sk = key.wk (DVE mult+reduce) -> softmax ->
p_row -> partition_broadcast (2 halves) -> 32 MiB att stream.
val load, o = p @ val, and out_part writes overlap the stream.
"""

from contextlib import ExitStack

import numpy as np

B, LQ, LK, D = 4, 4096, 4096, 128
LQ_HALF = LQ // 2
R = LK // 128  # 32 rows per partition in the contiguous load
ROWBLK = LQ_HALF // 128  # 16 output row blocks per core
N_CORES = 8

# Use fp16 for the sk = key @ w_k arithmetic (2x DVE throughput; ~5e-4
# relative error on p, far inside the 2e-2 gate). f32 otherwise.
FP16_SK = True

# Number of column-splits of the p broadcast / att stream start
N_BCAST_SPLITS = 4

_CACHE: dict = {}


def _build_graph():
    import concourse.bass as bass
    import concourse.mybir as mybir
    import concourse.tile as tile
    from concourse import bacc, bass_isa
    from concourse.tile import add_dep_helper

    F32 = mybir.dt.float32
    CDT = mybir.dt.float16 if FP16_SK else F32

    nc = bacc.Bacc("TRN2", target_bir_lowering=False, debug=False)
    keyb = nc.dram_tensor("keyb", [LK, D], F32, kind="ExternalInput")
    valb = nc.dram_tensor("valb", [LK, D], F32, kind="ExternalInput")
    wk = nc.dram_tensor("wk", [D, 1], F32, kind="ExternalInput")
    att_part = nc.dram_tensor("att_part", [LQ_HALF, LK], F32, kind="ExternalOutput")
    out_part = nc.dram_tensor("out_part", [LQ_HALF, D], F32, kind="ExternalOutput")

    with ExitStack() as ctx:
        tc = ctx.enter_context(tile.TileContext(nc))
        singles = ctx.enter_context(tc.tile_pool(name="singles", bufs=1))
        psum_small = ctx.enter_context(
            tc.tile_pool(name="psum_small", bufs=1, space="PSUM")
        )

        # ---- critical path: key load + wk broadcast
        # wkb[p, d] = wk[d]  (tiny partition-broadcast DMA from DRAM), on the
        # scalar ring so it isn't queued behind key's 2 MiB on the sync ring.
        wkb = singles.tile([128, D], CDT)
        wk_flat = wk.ap()
        wk_bcast = bass.AP(tensor=wk_flat.tensor, offset=0, ap=[[0, 128], [1, D]])
        if FP16_SK:
            nc.gpsimd.dma_start(out=wkb, in_=wk_bcast)  # SWDGE casts f32->fp16
        else:
            nc.scalar.dma_start(out=wkb, in_=wk_bcast)

        # key_sb[p, r, d] = keyb[p*R + r, d]: contiguous 16 KiB per partition
        if FP16_SK:
            key_sb = singles.tile([128, R, D], CDT)
            nc.gpsimd.dma_start(  # SWDGE: casts f32 -> fp16 in flight
                out=key_sb, in_=keyb.ap().rearrange("(p r) d -> p r d", p=128)
            )
        else:
            key_sb = singles.tile([128, R, D], F32)
            nc.sync.dma_start(
                out=key_sb, in_=keyb.ap().rearrange("(p r) d -> p r d", p=128)
            )

        # wkb_full[p, r, d] = wk[d], materialized during the key DMA
        wkb_full = singles.tile([128, R, D], CDT)
        wkb_ap = wkb[:]
        wkb3 = bass.AP(
            tensor=wkb_ap.tensor,
            offset=wkb_ap.offset,
            ap=[wkb_ap.ap[0], [0, R], wkb_ap.ap[1]],
        )
        nc.vector.tensor_copy(wkb_full, wkb3)

        # ---- sk[p, r] = sum_d key_sb[p, r, d] * wk[d]
        prod = singles.tile([128, R, D], CDT)
        nc.vector.tensor_mul(prod, key_sb, wkb_full)
        sk_all = singles.tile([128, R], CDT)
        if FP16_SK:
            # fp16 dot-product accumulation: |sk| <= ~4, fp16 noise ~1e-3,
            # p relative error ~1e-3 — far inside the 2e-2 gate. Keeping the
            # destination 2-byte is what enables the DVE 2x perf mode.
            with nc.allow_low_precision(reason="fp16 sk accumulation, ~1e-3 p err"):
                red_inst = nc.vector.reduce_sum(
                    out=sk_all, in_=prod, axis=mybir.AxisListType.X
                )
        else:
            red_inst = nc.vector.reduce_sum(
                out=sk_all, in_=prod, axis=mybir.AxisListType.X
            )

        # ---- softmax over the whole [128, R] tile
        m1 = singles.tile([128, 1], F32)
        nc.vector.reduce_max(out=m1, in_=sk_all, axis=mybir.AxisListType.X)
        m_all = singles.tile([128, 1], F32)
        nc.gpsimd.partition_all_reduce(
            m_all, m1, channels=128, reduce_op=bass_isa.ReduceOp.max
        )
        nm = singles.tile([128, 1], F32)
        nc.vector.tensor_scalar_mul(nm, m_all, -1.0)

        e_all = singles.tile([128, R], F32)
        s1 = singles.tile([128, 1], F32)
        nc.scalar.activation(
            out=e_all,
            in_=sk_all,
            func=mybir.ActivationFunctionType.Exp,
            bias=nm,
            scale=1.0,
            accum_out=s1,
        )
        s_all = singles.tile([128, 1], F32)
        nc.gpsimd.partition_all_reduce(
            s_all, s1, channels=128, reduce_op=bass_isa.ReduceOp.add
        )
        rinv = singles.tile([128, 1], F32)
        nc.vector.reciprocal(rinv, s_all)
        p_all = singles.tile([128, R], F32)
        nc.vector.tensor_scalar_mul(p_all, e_all, rinv)

        # ---- p_row [1, LK]: partition-major flatten of p_all IS j order
        p_row = singles.tile([1, LK], F32)
        prow_dma = nc.sync.dma_start(out=p_row, in_=p_all)

        # ---- p_rep [128, LK] in column splits; att stream starts after
        # the first split's broadcast
        p_rep = singles.tile([128, LK], F32)
        SPLIT = LK // N_BCAST_SPLITS
        att_ap = att_part.ap()
        last_pb = None
        for hh in range(N_BCAST_SPLITS):
            cs = slice(hh * SPLIT, (hh + 1) * SPLIT)
            last_pb = nc.gpsimd.partition_broadcast(p_rep[:, cs], p_row[:, cs])
            for k in range(ROWBLK):
                nc.sync.dma_start(
                    out=att_ap[k * 128 : (k + 1) * 128, cs], in_=p_rep[:, cs]
                )

        # ---- everything below overlaps the att stream
        # val load on the scalar HWDGE ring (so the gpsimd queue never waits
        # on its completion), held until the tiny p_row DMA is through the
        # shared SDMA engines (val packets would delay it by ~4 us)
        val_sb = singles.tile([128, R, D], F32)
        vdma = nc.scalar.dma_start(
            out=val_sb, in_=valb.ap().rearrange("(p r) d -> p r d", p=128)
        )
        add_dep_helper(
            getattr(vdma, "ins", vdma),
            getattr(prow_dma, "ins", prow_dma),
            reason="delay val load off key/p_row's SDMA window",
        )

        # o = p @ val: psum_o[1, D] accumulated over R column-tiles
        psum_o = psum_small.tile([1, D], F32)
        for r in range(R):
            nc.tensor.matmul(
                psum_o,
                lhsT=p_all[:, r : r + 1],
                rhs=val_sb[:, r, :],
                start=(r == 0),
                stop=(r == R - 1),
            )
        o_row = singles.tile([1, D], F32)
        nc.scalar.copy(o_row, psum_o)

        # o_rep [128, D]: broadcast o to all partitions (explicitly ordered
        # after the p_rep broadcasts so it can't head-of-line block them)
        o_rep = singles.tile([128, D], F32)
        opb = nc.gpsimd.partition_broadcast(o_rep, o_row)
        add_dep_helper(
            getattr(opb, "ins", opb),
            getattr(last_pb, "ins", last_pb),
            reason="o_rep broadcast must not precede p_rep broadcasts",
        )

        # out_part writes (16 x 64 KiB) on the scalar HWDGE ring
        out_ap = out_part.ap()
        for k in range(ROWBLK):
            nc.scalar.dma_start(out=out_ap[k * 128 : (k + 1) * 128, :], in_=o_rep)

    nc.compile()
    return nc


def _get_graph():
    if "nc" not in _CACHE:
        _CACHE["nc"] = _build_graph()
    return _CACHE["nc"]


def kernel(qry=None, key=None, val=None, w_q=None, w_k=None, **_ignored):
    """Full (unsharded) inputs in, full outputs out.

    Returns (out, att) matching reference.reference(). qry/w_q are
    mathematically irrelevant (softmax shift invariance) and unused.
    """
    from concourse.bass_utils import run_bass_kernel_spmd

    key = np.ascontiguousarray(np.asarray(key, dtype=np.float32))
    val = np.ascontiguousarray(np.asarray(val, dtype=np.float32))
    w_k = np.ascontiguousarray(np.asarray(w_k, dtype=np.float32))

    nc = _get_graph()
    in_maps = []
    for c in range(N_CORES):
        b = c // 2
        in_maps.append({"keyb": key[b], "valb": val[b], "wk": w_k})

    res = run_bass_kernel_spmd(nc, in_maps, core_ids=list(range(N_CORES)))

    out = np.empty((B, LQ, D), np.float32)
    att = np.empty((B, LQ, LK), np.float32)
    for c in range(N_CORES):
        b, h = c // 2, c % 2
        att[b, h * LQ_HALF : (h + 1) * LQ_HALF, :] = res.results[c]["att_part"]
        out[b, h * LQ_HALF : (h + 1) * LQ_HALF, :] = res.results[c]["out_part"]
    return out, att
